# revision 25
# baseline (speedup 1.0000x reference)
"""TRN2 Bass kernel for nn_DEAM_5076651343977 (dense_transformer).

Computation (per sample):
    d  = avg_pool8(diff)                      [C, 32, 32] -> [C, N=1024]
    q  = Wq d + bq ; k = Wk d + bk
    E[n,m] = sum_c q[c,n] k[c,m] * C^-0.5
    attn = softmax_m(E)
    v  = Wv avg_pool8(x) + bv
    out_small[c,n] = sum_m v[c,m] attn[n,m]
    out = repeat8(out_small) + x

Sharding: pure data parallel, one sample per NeuronCore (B=8 over 8 cores).

The kernel is HBM-bandwidth-bound (per-core DMA ceiling ~425 GB/s, all
queues share the 16 SDMA engines), so inputs/output are staged in DRAM as
bf16: 24 MiB/core of traffic instead of 48 MiB.  The rel-err budget (2e-2
vs a ~5.4 denominator) dwarfs bf16 rounding (~4e-3 observed).

Per-core layout: partitions p = s*64 + c with s = hp%2 (h-block parity),
free = hpp*2048 + r*256 + w  (h = (2*hpp+s)*8 + r, w = wp*8 + i).
Blocks are processed in groups of 2 (one attention m-tile of 128 = 2
blocks), with one DMA per s-half per group (1 MiB transfers).

Engine budget notes (all hide under the ~57us DMA spine):
 - Pooling runs as a tensor_tensor add tree (bf16 = 2 elem/cycle on DVE)
   over the r dim, then one 1x tensor_reduce over the innermost i dim --
   a single tensor_reduce would be 1x over the full input and become the
   bottleneck.
 - avg-pool 1/64 + conv bias are folded into augmented weights (K=65,
   ones row appended to pooled activations).  Softmax max-subtraction is
   skipped (|0.125*E| << 1 for pooled unit-variance inputs); the softmax
   denominator comes free as a 65th output row of the out_small matmul
   (ones column in v^T).
 - The final upsample+residual is a DVE tensor_tensor add at 2x: the
   small operand is pre-expanded along w (os_w) on the scalar engine so
   only the r-broadcast uses a zero stride (outer dim keeps 2x mode).
"""
import numpy as np
import ml_dtypes

import concourse.bass as bass
import concourse.mybir as mybir
from concourse import bacc
from concourse.tile import TileContext
from concourse.bass_utils import run_bass_kernel_spmd

f32 = mybir.dt.float32
bf16 = mybir.dt.bfloat16

B, C, H, W = 8, 64, 256, 256
DS = 8
HW = H * W            # 65536
NB = 16               # h-pair blocks per sample
BLK = 2048            # free elems per block per partition (8 rows x 256)
NG = 8                # groups of 2 blocks (one attention m-tile each)
GRP = 2 * BLK         # 4096

_cache = {}


def _xpack2_dma(nc, dst_ap, dram, g, store=False):
    """Move block group g (blocks 2g, 2g+1) between DRAM [c, h*w] and the
    (s,c)-packed SBUF tile.  One DMA per s-half: partition walk is a
    single level (stride HW), free side is 2 runs of 4 KiB per partition.
    """
    for s in range(2):
        ap = bass.AP(dram, (2 * g) * 2 * BLK + s * BLK,
                     [[HW, C], [2 * BLK, 2], [1, BLK]])
        half = dst_ap[s * 64:(s + 1) * 64, :]
        if store:
            nc.sync.dma_start(ap, half)
        else:
            nc.sync.dma_start(half, ap)


def _pool_tree(nc, pools, src_ap, s1, s2, s3, pooled, g):
    """Avg-pool (sum) of a 2-block group: r-dim via bf16 TT adds (2x mode),
    then i-dim via one 1x tensor_reduce into f32 pooled[:, g*64:g*64+64]."""
    TT = nc.vector.tensor_tensor
    ADD = mybir.AluOpType.add

    def v(tile_ap, dims):
        a0 = tile_ap
        return bass.AP(a0.tensor, a0.offset, [list(a0.ap[0])] + dims)

    # level 1: r0..3 + r4..7  -> s1 [p, (blk2)(r'4)(w256)]
    TT(v(s1, [[1024, 2], [1, 1024]]),
       v(src_ap, [[2048, 2], [1, 1024]]),
       bass.AP(src_ap.tensor, src_ap.offset + 1024,
               [list(src_ap.ap[0]), [2048, 2], [1, 1024]]),
       ADD)
    # level 2 -> s2 [p, (blk2)(r''2)(w256)]
    TT(v(s2, [[512, 2], [1, 512]]),
       v(s1, [[1024, 2], [1, 512]]),
       bass.AP(s1.tensor, s1.offset + 512, [list(s1.ap[0]), [1024, 2], [1, 512]]),
       ADD)
    # level 3 -> s3 [p, (blk2)(w256)]
    TT(v(s3, [[256, 2], [1, 256]]),
       v(s2, [[512, 2], [1, 256]]),
       bass.AP(s2.tensor, s2.offset + 256, [list(s2.ap[0]), [512, 2], [1, 256]]),
       ADD)
    # i-reduce (1x) -> pooled bf16 [p, (blk2)(wp32)]
    with nc.allow_low_precision(reason="bf16 pooled sums; rel err << 2e-2"):
        nc.vector.tensor_reduce(
            v(pooled[:, g * 64:(g + 1) * 64], [[32, 2], [1, 32]]),
            v(s3, [[256, 2], [8, 32], [1, 8]]),
            axis=mybir.AxisListType.X, op=ADD)


def _repack_dma(nc, dst, s, src_half):
    """SB->SB repack pooled [s*64+c, t*64+blk*32+wp] -> d_aug[c, j] where
    columns use the s-major tile order j = t*128 + s*64 + blk*32 + wp.
    This makes every matmul lhsT slice contiguous (the verifier allows only
    one free dim on matmul operands) and matches the vps partition order.
    On the scalar HWDGE queue: RTL descriptor generation, so the many small
    descriptors don't serialize on the Q7 (SWDGE)."""
    ap = bass.AP(dst.tensor, dst.offset + s * 64,
                 [list(dst.ap[0]), [128, 8], [1, 64]])
    nc.scalar.dma_start(ap, src_half)


def _emit(nc, tc, pools, drams):
    big, stream, tree, small, attnp, psA, psE, psO = pools
    x_d, diff_d, wq_d, wk_d, wv_d, bv_d, out_d = drams
    Exp = mybir.ActivationFunctionType.Exp
    Copy = mybir.ActivationFunctionType.Copy

    wq = small.tile([65, 64], bf16, name="wq_sb")
    wk = small.tile([65, 64], bf16, name="wk_sb")
    wv = small.tile([128, 64], bf16, name="wv_sb")
    bv_sb = small.tile([1, 64], bf16, name="bv_sb")
    nc.scalar.dma_start(wq, wq_d[:, :])
    nc.scalar.dma_start(wk, wk_d[:, :])
    nc.scalar.dma_start(wv, wv_d[:, :])
    nc.scalar.dma_start(bv_sb, bv_d[:, :])

    x_sb = big.tile([128, NB * BLK], bf16, name="x_sb")
    pooled_x = small.tile([128, 512], bf16, name="pooled_x")
    pooled_f = small.tile([128, 512], bf16, name="pooled_f")

    d_aug = small.tile([65, 1024], bf16, name="d_aug")
    nc.gpsimd.memset(d_aug[64:65, :], 1.0)
    vT = small.tile([128, 8 * 65], bf16, name="vT")
    nc.gpsimd.memset(vT[:, :], 1.0)
    ones1b = small.tile([1, 128], bf16, name="ones1b")
    nc.gpsimd.memset(ones1b[:, :], 1.0)
    q_sb = small.tile([64, 1024], bf16, name="q_sb")
    k_sb = small.tile([64, 1024], bf16, name="k_sb")
    out_ps = psO.tile([65, 1024], f32, name="out_ps")

    # ---- phase 1: stream diff in 2-block groups, pool, then q,k ----
    for g in range(NG):
        db = stream.tile([128, GRP], bf16, name="db", tag="blk")
        _xpack2_dma(nc, db, diff_d, g)
        s1 = tree.tile([128, 2048], bf16, name="s1", tag="s1")
        s2 = tree.tile([128, 1024], bf16, name="s2", tag="s2")
        s3 = tree.tile([128, 512], bf16, name="s3", tag="s3")
        _pool_tree(nc, pools, db[:, :], s1, s2, s3, pooled_f, g)
    for s in range(2):
        _repack_dma(nc, d_aug[0:64, :], s, pooled_f[s * 64:(s + 1) * 64, :])
    for (w_t, dst) in ((wq, q_sb), (wk, k_sb)):
        ps = psA.tile([64, 1024], f32, name="qk_ps", tag="psa")
        for ch in range(2):
            nc.tensor.matmul(ps[:, ch * 512:(ch + 1) * 512], w_t[:, :],
                             d_aug[:, ch * 512:(ch + 1) * 512],
                             start=True, stop=True)
        nc.scalar.copy(dst[:, :], ps[:, :])

    # ---- phase 2: stream x; attention paced per m-tile t (= group) ----
    # m-tiles are ordered s-major (m_local = s*64 + blk*32 + wp) so that v
    # comes straight from pooled_x with two K=64 matmuls -- no SB->SB
    # repack on the per-tile critical path.  k_sb supplies the matching
    # order via a permuted 3-level lhsT access pattern; the attention sum
    # over m is order-invariant, so q/n ordering is unaffected.
    for t in range(NG):
        xg = x_sb[:, t * GRP:(t + 1) * GRP]
        _xpack2_dma(nc, xg, x_d, t)
        s1 = tree.tile([128, 2048], bf16, name="s1", tag="s1")
        s2 = tree.tile([128, 1024], bf16, name="s2", tag="s2")
        s3 = tree.tile([128, 512], bf16, name="s3", tag="s3")
        _pool_tree(nc, pools, xg, s1, s2, s3, pooled_x, t)
        vps = psA.tile([128, 64], f32, name="vps", tag="psa")
        # conv bias: vps[m, c] starts at bv[c] (rank-1), then += Wv^T d
        nc.tensor.matmul(vps[:, :], ones1b[:, :], bv_sb[:, :],
                         start=True, stop=False)
        for s in range(2):
            half = slice(s * 64, (s + 1) * 64)
            nc.tensor.matmul(vps[half, :], pooled_x[half, t * 64:(t + 1) * 64],
                             wv[half, :], start=False, stop=True)
        nc.scalar.copy(vT[:, t * 65:t * 65 + 64], vps[:, :])
        et = psE.tile([128, 1024], f32, name="et", tag="et")
        for ch in range(2):
            nc.tensor.matmul(et[:, ch * 512:(ch + 1) * 512],
                             k_sb[:, t * 128:(t + 1) * 128],
                             q_sb[:, ch * 512:(ch + 1) * 512],
                             start=True, stop=True)
        at = attnp.tile([128, 1024], bf16, name="at", tag="at")
        for ch in range(2):
            csl = slice(ch * 512, (ch + 1) * 512)
            nc.scalar.activation(at[:, csl], et[:, csl], Exp, scale=0.125)
            nc.tensor.matmul(out_ps[:, csl], vT[:, t * 65:(t + 1) * 65],
                             at[:, csl], start=(t == 0), stop=(t == 7))

    # ---- phase 3+4: normalize (chunked by n so stores start early),
    #      pack to (s,c), expand along w on ACT, add + store ----
    den_sb = small.tile([1, 1024], bf16, name="den_sb")
    ones1 = small.tile([1, 64], bf16, name="ones1")
    nc.gpsimd.memset(ones1[:, :], 1.0)
    rb_sb = small.tile([64, 1024], f32, name="rb_sb")
    osn = small.tile([64, 1024], bf16, name="osn")
    os2 = small.tile([128, 512], bf16, name="os2")
    os_w = small.tile([128, NB * 256], bf16, name="os_w")

    for q in range(4):          # n-chunk q covers n in [256q, 256q+256)
        nsl = slice(256 * q, 256 * (q + 1))
        nc.scalar.copy(den_sb[:, nsl], out_ps[64:65, nsl])
        rb_ps = psA.tile([64, 256], f32, name="rb_ps", tag="psa")
        nc.tensor.matmul(rb_ps[:, :], ones1[:, :], den_sb[:, nsl],
                         start=True, stop=True)
        nc.vector.reciprocal_approx_fast(rb_sb[:, nsl], rb_ps[:, :])
        nc.vector.tensor_tensor(osn[:, nsl], out_ps[0:64, nsl],
                                rb_sb[:, nsl], mybir.AluOpType.mult)
        # pack: os2[s*64+c, hpp*32+wp] <- osn[c, j] with the s-major column
        # order j = t*128 + s*64 + blk*32 + wp (hpp = 2t + blk).  Chunk q
        # covers tiles t in {2q, 2q+1} -> os2 cols [128q, 128q+128).
        for s in range(2):
            src = bass.AP(osn.tensor, osn.offset + 256 * q + s * 64,
                          [list(osn.ap[0]), [128, 2], [1, 64]])
            nc.scalar.dma_start(os2[s * 64:(s + 1) * 64, 128 * q:128 * (q + 1)],
                                src)
        for g in (2 * q, 2 * q + 1):
            nc.scalar.activation(
                os_w[:, 512 * g:512 * (g + 1)],
                bass.AP(os2.tensor, os2.offset + 64 * g,
                        [list(os2.ap[0]), [32, 2], [1, 32], [0, 8]]),
                Copy)
            xg = x_sb[:, g * GRP:(g + 1) * GRP]
            ob = stream.tile([128, GRP], bf16, name="ob", tag="blk")

            def v3(ap, dims):
                return bass.AP(ap.tensor, ap.offset, [list(ap.ap[0])] + dims)

            nc.vector.tensor_tensor(
                v3(ob[:, :], [[2048, 2], [256, 8], [1, 256]]),
                v3(xg, [[2048, 2], [256, 8], [1, 256]]),
                bass.AP(os_w.tensor, os_w.offset + g * 512,
                        [list(os_w.ap[0]), [256, 2], [0, 8], [1, 256]]),
                mybir.AluOpType.add)
            _xpack2_dma(nc, ob, out_d, g, store=True)


def _build(dup=1):
    nc = bacc.Bacc("TRN2", target_bir_lowering=False, debug=False, num_devices=8)

    x_d = nc.dram_tensor("x", [C, HW], bf16, kind="ExternalInput")
    diff_d = nc.dram_tensor("diff", [C, HW], bf16, kind="ExternalInput")
    wq_d = nc.dram_tensor("wq", [65, 64], bf16, kind="ExternalInput")
    wk_d = nc.dram_tensor("wk", [65, 64], bf16, kind="ExternalInput")
    wv_d = nc.dram_tensor("wv", [128, 64], bf16, kind="ExternalInput")
    bv_d = nc.dram_tensor("bv", [1, 64], bf16, kind="ExternalInput")
    out_d = nc.dram_tensor("out", [C, HW], bf16, kind="ExternalOutput")
    drams = (x_d, diff_d, wq_d, wk_d, wv_d, bv_d, out_d)

    with TileContext(nc) as tc:
        with tc.tile_pool(name="big", bufs=1) as big, \
             tc.tile_pool(name="stream", bufs=3) as stream, \
             tc.tile_pool(name="tree", bufs=2) as tree, \
             tc.tile_pool(name="small", bufs=1) as small, \
             tc.tile_pool(name="attn", bufs=2) as attnp, \
             tc.tile_pool(name="psA", bufs=1, space="PSUM") as psA, \
             tc.tile_pool(name="psE", bufs=2, space="PSUM") as psE, \
             tc.tile_pool(name="psO", bufs=1, space="PSUM") as psO:
            pools = (big, stream, tree, small, attnp, psA, psE, psO)
            for rep in range(dup):
                if rep:
                    tc.strict_bb_all_engine_barrier()
                _emit(nc, tc, pools, drams)

    nc.compile()
    return nc


def make_in_maps(inputs):
    bf = ml_dtypes.bfloat16
    x = np.asarray(inputs["x"], dtype=np.float32).reshape(B, C, HW).astype(bf)
    diff = (np.asarray(inputs["diff"], dtype=np.float32)
            .reshape(B, C, HW).astype(bf))
    # fold avg-pool 1/64 into the weights; append bias row (K=65 aug trick)
    inv = 1.0 / (DS * DS)
    wq_aug = np.concatenate(
        [np.asarray(inputs["Wq"]).T * inv, np.asarray(inputs["bq"])[None, :]], 0)
    wk_aug = np.concatenate(
        [np.asarray(inputs["Wk"]).T * inv, np.asarray(inputs["bk"])[None, :]], 0)
    wvt = np.asarray(inputs["Wv"]).T * inv
    wv2 = np.concatenate([wvt, wvt], 0)      # duplicated per s-half partition
    wq_aug = np.ascontiguousarray(wq_aug, dtype=np.float32).astype(bf)
    wk_aug = np.ascontiguousarray(wk_aug, dtype=np.float32).astype(bf)
    wv2 = np.ascontiguousarray(wv2, dtype=np.float32).astype(bf)
    bv = (np.ascontiguousarray(np.asarray(inputs["bv"])[None, :],
                               dtype=np.float32).astype(bf))
    return [
        {
            "x": x[b],
            "diff": diff[b],
            "wq": wq_aug, "wk": wk_aug, "wv": wv2, "bv": bv,
        }
        for b in range(B)
    ]


def kernel(x, diff, Wq, bq, Wk, bk, Wv, bv):
    if "nc" not in _cache:
        _cache["nc"] = _build()
    nc = _cache["nc"]

    in_maps = make_in_maps(dict(x=x, diff=diff, Wq=Wq, bq=bq, Wk=Wk, bk=bk,
                                Wv=Wv, bv=bv))
    res = run_bass_kernel_spmd(nc, in_maps, list(range(B)))
    out = np.stack([np.asarray(res.results[b]["out"])
                    .astype(np.float32).reshape(C, H, W) for b in range(B)])
    return out


if __name__ == "__main__":
    rng = np.random.default_rng(0)
    xs = rng.standard_normal((B, C, H, W), dtype=np.float32)
    ds = rng.standard_normal((B, C, H, W), dtype=np.float32)
    sc = 1.0 / np.sqrt(C)
    args = dict(
        x=xs, diff=ds,
        Wq=rng.standard_normal((C, C), dtype=np.float32) * sc,
        bq=rng.standard_normal(C, dtype=np.float32) * 0.01,
        Wk=rng.standard_normal((C, C), dtype=np.float32) * sc,
        bk=rng.standard_normal(C, dtype=np.float32) * 0.01,
        Wv=rng.standard_normal((C, C), dtype=np.float32) * sc,
        bv=rng.standard_normal(C, dtype=np.float32) * 0.01,
    )
    out = kernel(**args)
    print("kernel ran, out shape", out.shape, out.dtype)


# revision 26
# speedup vs baseline: 1.4283x; 1.4283x over previous
"""TRN2 Bass kernel for nn_DEAM_5076651343977 (dense_transformer).

Computation (per sample):
    d  = avg_pool8(diff)                      [C, 32, 32] -> [C, N=1024]
    q  = Wq d + bq ; k = Wk d + bk
    E[n,m] = sum_c q[c,n] k[c,m] * C^-0.5
    attn = softmax_m(E)
    v  = Wv avg_pool8(x) + bv
    out_small[c,n] = sum_m v[c,m] attn[n,m]
    out = repeat8(out_small) + x

Sharding: pure data parallel, one sample per NeuronCore (B=8 over 8 cores).

The kernel is HBM-bandwidth-bound (per-core DMA ceiling ~425 GB/s, all
queues share the 16 SDMA engines), so inputs/output are staged in DRAM as
bf16: 24 MiB/core of traffic instead of 48 MiB.  The rel-err budget (2e-2
vs a ~5.4 denominator) dwarfs bf16 rounding (~4e-3 observed).

Per-core layout: partitions p = s*64 + c with s = hp%2 (h-block parity),
free = hpp*2048 + r*256 + w  (h = (2*hpp+s)*8 + r, w = wp*8 + i).
Blocks are processed in groups of 2 (one attention m-tile of 128 = 2
blocks), with one DMA per s-half per group (1 MiB transfers).

Engine budget notes (all hide under the ~57us DMA spine):
 - Pooling runs as a tensor_tensor add tree (bf16 = 2 elem/cycle on DVE)
   over the r dim, then one 1x tensor_reduce over the innermost i dim --
   a single tensor_reduce would be 1x over the full input and become the
   bottleneck.
 - avg-pool 1/64 + conv bias are folded into augmented weights (K=65,
   ones row appended to pooled activations).  Softmax max-subtraction is
   skipped (|0.125*E| << 1 for pooled unit-variance inputs); the softmax
   denominator comes free as a 65th output row of the out_small matmul
   (ones column in v^T).
 - The final upsample+residual is a DVE tensor_tensor add at 2x: the
   small operand is pre-expanded along w (os_w) on the scalar engine so
   only the r-broadcast uses a zero stride (outer dim keeps 2x mode).
"""
import numpy as np
import ml_dtypes

import concourse.bass as bass
import concourse.mybir as mybir
from concourse import bacc
from concourse.tile import TileContext
from concourse.bass_utils import run_bass_kernel_spmd

f32 = mybir.dt.float32
bf16 = mybir.dt.bfloat16

B, C, H, W = 8, 64, 256, 256
DS = 8
HW = H * W            # 65536
NB = 16               # h-pair blocks per sample
BLK = 2048            # free elems per block per partition (8 rows x 256)
NG = 8                # groups of 2 blocks (one attention m-tile each)
GRP = 2 * BLK         # 4096

_cache = {}


def _xpack2_dma(nc, dst_ap, dram, g, store=False):
    """Move block group g (blocks 2g, 2g+1) between DRAM [c, h*w] and the
    (s,c)-packed SBUF tile.  One DMA per s-half: partition walk is a
    single level (stride HW), free side is 2 runs of 4 KiB per partition.
    """
    for s in range(2):
        ap = bass.AP(dram, (2 * g) * 2 * BLK + s * BLK,
                     [[HW, C], [2 * BLK, 2], [1, BLK]])
        half = dst_ap[s * 64:(s + 1) * 64, :]
        if store:
            nc.sync.dma_start(ap, half)
        else:
            nc.sync.dma_start(half, ap)


def _pool_tree(nc, pools, src_ap, s1, s2, s3, pooled, g):
    """Avg-pool (sum) of a 2-block group: r-dim via bf16 TT adds (2x mode),
    then i-dim via one 1x tensor_reduce into f32 pooled[:, g*64:g*64+64]."""
    TT = nc.vector.tensor_tensor
    ADD = mybir.AluOpType.add

    def v(tile_ap, dims):
        a0 = tile_ap
        return bass.AP(a0.tensor, a0.offset, [list(a0.ap[0])] + dims)

    # level 1: r0..3 + r4..7  -> s1 [p, (blk2)(r'4)(w256)]
    TT(v(s1, [[1024, 2], [1, 1024]]),
       v(src_ap, [[2048, 2], [1, 1024]]),
       bass.AP(src_ap.tensor, src_ap.offset + 1024,
               [list(src_ap.ap[0]), [2048, 2], [1, 1024]]),
       ADD)
    # level 2 -> s2 [p, (blk2)(r''2)(w256)]
    TT(v(s2, [[512, 2], [1, 512]]),
       v(s1, [[1024, 2], [1, 512]]),
       bass.AP(s1.tensor, s1.offset + 512, [list(s1.ap[0]), [1024, 2], [1, 512]]),
       ADD)
    # level 3 -> s3 [p, (blk2)(w256)]
    TT(v(s3, [[256, 2], [1, 256]]),
       v(s2, [[512, 2], [1, 256]]),
       bass.AP(s2.tensor, s2.offset + 256, [list(s2.ap[0]), [512, 2], [1, 256]]),
       ADD)
    # i-reduce (1x) -> pooled bf16 [p, (blk2)(wp32)]
    with nc.allow_low_precision(reason="bf16 pooled sums; rel err << 2e-2"):
        nc.vector.tensor_reduce(
            v(pooled[:, g * 64:(g + 1) * 64], [[32, 2], [1, 32]]),
            v(s3, [[256, 2], [8, 32], [1, 8]]),
            axis=mybir.AxisListType.X, op=ADD)


def _repack_dma(nc, dst, s, src_half):
    """SB->SB repack pooled [s*64+c, t*64+blk*32+wp] -> d_aug[c, j] where
    columns use the s-major tile order j = t*128 + s*64 + blk*32 + wp.
    This makes every matmul lhsT slice contiguous (the verifier allows only
    one free dim on matmul operands) and matches the vps partition order.
    On the scalar HWDGE queue: RTL descriptor generation, so the many small
    descriptors don't serialize on the Q7 (SWDGE)."""
    ap = bass.AP(dst.tensor, dst.offset + s * 64,
                 [list(dst.ap[0]), [128, 8], [1, 64]])
    nc.scalar.dma_start(ap, src_half)


def _emit(nc, tc, pools, drams):
    big, stream, tree, small, attnp, psA, psE, psO = pools
    x_d, diff_d, wq_d, wk_d, wv_d, bv_d, out_d = drams
    Exp = mybir.ActivationFunctionType.Exp
    Copy = mybir.ActivationFunctionType.Copy

    wq = small.tile([65, 64], bf16, name="wq_sb")
    wk = small.tile([65, 64], bf16, name="wk_sb")
    wv = small.tile([128, 64], bf16, name="wv_sb")
    bv_sb = small.tile([1, 64], bf16, name="bv_sb")
    nc.scalar.dma_start(wq, wq_d[:, :])
    nc.scalar.dma_start(wk, wk_d[:, :])
    nc.scalar.dma_start(wv, wv_d[:, :])
    nc.scalar.dma_start(bv_sb, bv_d[:, :])

    x_sb = big.tile([128, NB * BLK], bf16, name="x_sb")
    pooled_x = small.tile([128, 512], bf16, name="pooled_x")
    pooled_f = small.tile([128, 512], bf16, name="pooled_f")

    d_aug = small.tile([65, 1024], bf16, name="d_aug")
    nc.gpsimd.memset(d_aug[64:65, :], 1.0)
    vT = small.tile([128, 8 * 65], bf16, name="vT")
    nc.gpsimd.memset(vT[:, :], 1.0)
    ones1b = small.tile([1, 128], bf16, name="ones1b")
    nc.gpsimd.memset(ones1b[:, :], 1.0)
    q_sb = small.tile([64, 1024], bf16, name="q_sb")
    k_sb = small.tile([64, 1024], bf16, name="k_sb")
    out_ps = psO.tile([65, 1024], f32, name="out_ps")

    # ---- phase 1: stream diff in 2-block groups, pool, then q,k ----
    for g in range(NG):
        db = stream.tile([128, GRP], bf16, name="db", tag="blk")
        _xpack2_dma(nc, db, diff_d, g)
        s1 = tree.tile([128, 2048], bf16, name="s1", tag="s1")
        s2 = tree.tile([128, 1024], bf16, name="s2", tag="s2")
        s3 = tree.tile([128, 512], bf16, name="s3", tag="s3")
        _pool_tree(nc, pools, db[:, :], s1, s2, s3, pooled_f, g)
    for s in range(2):
        _repack_dma(nc, d_aug[0:64, :], s, pooled_f[s * 64:(s + 1) * 64, :])
    for (w_t, dst) in ((wq, q_sb), (wk, k_sb)):
        ps = psA.tile([64, 1024], f32, name="qk_ps", tag="psa")
        for ch in range(2):
            nc.tensor.matmul(ps[:, ch * 512:(ch + 1) * 512], w_t[:, :],
                             d_aug[:, ch * 512:(ch + 1) * 512],
                             start=True, stop=True)
        nc.scalar.copy(dst[:, :], ps[:, :])

    # ---- phase 2: stream x; attention paced per m-tile t (= group) ----
    # m-tiles are ordered s-major (m_local = s*64 + blk*32 + wp) so that v
    # comes straight from pooled_x with two K=64 matmuls -- no SB->SB
    # repack on the per-tile critical path.  k_sb supplies the matching
    # order via a permuted 3-level lhsT access pattern; the attention sum
    # over m is order-invariant, so q/n ordering is unaffected.
    for t in range(NG):
        xg = x_sb[:, t * GRP:(t + 1) * GRP]
        if t < NG - 1:
            _xpack2_dma(nc, xg, x_d, t)
            s1 = tree.tile([128, 2048], bf16, name="s1", tag="s1")
            s2 = tree.tile([128, 1024], bf16, name="s2", tag="s2")
            s3 = tree.tile([128, 512], bf16, name="s3", tag="s3")
            _pool_tree(nc, pools, xg, s1, s2, s3, pooled_x, t)
        else:
            # last group: per-block DMAs + single-block trees so the first
            # block's pooling overlaps the second block's load -- shortens
            # the attention drain that gates the whole normalization tail.
            s1 = tree.tile([128, 2048], bf16, name="s1", tag="s1")
            s2 = tree.tile([128, 1024], bf16, name="s2", tag="s2")
            s3 = tree.tile([128, 512], bf16, name="s3", tag="s3")
            TT = nc.vector.tensor_tensor
            ADD = mybir.AluOpType.add
            for j in range(2):
                hpp = 2 * t + j
                xb = x_sb[:, hpp * BLK:(hpp + 1) * BLK]
                for s in range(2):
                    ap = bass.AP(x_d, hpp * 2 * BLK + s * BLK,
                                 [[HW, C], [1, BLK]])
                    nc.sync.dma_start(xb[s * 64:(s + 1) * 64, :], ap)
                TT(s1[:, j * 1024:(j + 1) * 1024], xb[:, 0:1024],
                   xb[:, 1024:2048], ADD)
                TT(s2[:, j * 512:(j + 1) * 512], s1[:, j * 1024:j * 1024 + 512],
                   s1[:, j * 1024 + 512:(j + 1) * 1024], ADD)
                TT(s3[:, j * 256:(j + 1) * 256], s2[:, j * 512:j * 512 + 256],
                   s2[:, j * 512 + 256:(j + 1) * 512], ADD)
            with nc.allow_low_precision(reason="bf16 pooled sums"):
                nc.vector.tensor_reduce(
                    bass.AP(pooled_x.tensor,
                            pooled_x.offset + t * 64,
                            [list(pooled_x.ap[0]), [32, 2], [1, 32]]),
                    bass.AP(s3.tensor, s3.offset,
                            [list(s3.ap[0]), [256, 2], [8, 32], [1, 8]]),
                    axis=mybir.AxisListType.X, op=ADD)
        vps = psA.tile([128, 64], f32, name="vps", tag="psa")
        # conv bias: vps[m, c] starts at bv[c] (rank-1), then += Wv^T d
        nc.tensor.matmul(vps[:, :], ones1b[:, :], bv_sb[:, :],
                         start=True, stop=False)
        for s in range(2):
            half = slice(s * 64, (s + 1) * 64)
            nc.tensor.matmul(vps[half, :], pooled_x[half, t * 64:(t + 1) * 64],
                             wv[half, :], start=False, stop=True)
        nc.scalar.copy(vT[:, t * 65:t * 65 + 64], vps[:, :])
        et = psE.tile([128, 1024], f32, name="et", tag="et")
        for ch in range(2):
            nc.tensor.matmul(et[:, ch * 512:(ch + 1) * 512],
                             k_sb[:, t * 128:(t + 1) * 128],
                             q_sb[:, ch * 512:(ch + 1) * 512],
                             start=True, stop=True)
        at = attnp.tile([128, 1024], bf16, name="at", tag="at")
        for ch in range(2):
            csl = slice(ch * 512, (ch + 1) * 512)
            nc.scalar.activation(at[:, csl], et[:, csl], Exp, scale=0.125)
            nc.tensor.matmul(out_ps[:, csl], vT[:, t * 65:(t + 1) * 65],
                             at[:, csl], start=(t == 0), stop=(t == 7))

    # ---- phase 3+4: normalize (chunked by n so stores start early),
    #      pack to (s,c), expand along w on ACT, add + store ----
    den_sb = small.tile([1, 1024], bf16, name="den_sb")
    ones1 = small.tile([1, 64], bf16, name="ones1")
    nc.gpsimd.memset(ones1[:, :], 1.0)
    rb_sb = small.tile([64, 1024], f32, name="rb_sb")
    osn = small.tile([64, 1024], bf16, name="osn")
    os2 = small.tile([128, 512], bf16, name="os2")
    os_w = small.tile([128, NB * 256], bf16, name="os_w")

    for q in range(4):          # n-chunk q covers n in [256q, 256q+256)
        nsl = slice(256 * q, 256 * (q + 1))
        nc.scalar.copy(den_sb[:, nsl], out_ps[64:65, nsl])
        rb_ps = psA.tile([64, 256], f32, name="rb_ps", tag="psa")
        nc.tensor.matmul(rb_ps[:, :], ones1[:, :], den_sb[:, nsl],
                         start=True, stop=True)
        nc.vector.reciprocal_approx_fast(rb_sb[:, nsl], rb_ps[:, :])
        nc.vector.tensor_tensor(osn[:, nsl], out_ps[0:64, nsl],
                                rb_sb[:, nsl], mybir.AluOpType.mult)
        # pack: os2[s*64+c, hpp*32+wp] <- osn[c, j] with the s-major column
        # order j = t*128 + s*64 + blk*32 + wp (hpp = 2t + blk).  Chunk q
        # covers tiles t in {2q, 2q+1} -> os2 cols [128q, 128q+128).
        for s in range(2):
            src = bass.AP(osn.tensor, osn.offset + 256 * q + s * 64,
                          [list(osn.ap[0]), [128, 2], [1, 64]])
            nc.scalar.dma_start(os2[s * 64:(s + 1) * 64, 128 * q:128 * (q + 1)],
                                src)
        for g in (2 * q, 2 * q + 1):
            nc.scalar.activation(
                os_w[:, 512 * g:512 * (g + 1)],
                bass.AP(os2.tensor, os2.offset + 64 * g,
                        [list(os2.ap[0]), [32, 2], [1, 32], [0, 8]]),
                Copy)
            xg = x_sb[:, g * GRP:(g + 1) * GRP]
            ob = stream.tile([128, GRP], bf16, name="ob", tag="blk")

            def v3(ap, dims):
                return bass.AP(ap.tensor, ap.offset, [list(ap.ap[0])] + dims)

            nc.vector.tensor_tensor(
                v3(ob[:, :], [[2048, 2], [256, 8], [1, 256]]),
                v3(xg, [[2048, 2], [256, 8], [1, 256]]),
                bass.AP(os_w.tensor, os_w.offset + g * 512,
                        [list(os_w.ap[0]), [256, 2], [0, 8], [1, 256]]),
                mybir.AluOpType.add)
            _xpack2_dma(nc, ob, out_d, g, store=True)


def _build(dup=1):
    nc = bacc.Bacc("TRN2", target_bir_lowering=False, debug=False, num_devices=8)

    x_d = nc.dram_tensor("x", [C, HW], bf16, kind="ExternalInput")
    diff_d = nc.dram_tensor("diff", [C, HW], bf16, kind="ExternalInput")
    wq_d = nc.dram_tensor("wq", [65, 64], bf16, kind="ExternalInput")
    wk_d = nc.dram_tensor("wk", [65, 64], bf16, kind="ExternalInput")
    wv_d = nc.dram_tensor("wv", [128, 64], bf16, kind="ExternalInput")
    bv_d = nc.dram_tensor("bv", [1, 64], bf16, kind="ExternalInput")
    out_d = nc.dram_tensor("out", [C, HW], bf16, kind="ExternalOutput")
    drams = (x_d, diff_d, wq_d, wk_d, wv_d, bv_d, out_d)

    with TileContext(nc) as tc:
        with tc.tile_pool(name="big", bufs=1) as big, \
             tc.tile_pool(name="stream", bufs=3) as stream, \
             tc.tile_pool(name="tree", bufs=2) as tree, \
             tc.tile_pool(name="small", bufs=1) as small, \
             tc.tile_pool(name="attn", bufs=2) as attnp, \
             tc.tile_pool(name="psA", bufs=1, space="PSUM") as psA, \
             tc.tile_pool(name="psE", bufs=2, space="PSUM") as psE, \
             tc.tile_pool(name="psO", bufs=1, space="PSUM") as psO:
            pools = (big, stream, tree, small, attnp, psA, psE, psO)
            for rep in range(dup):
                if rep:
                    tc.strict_bb_all_engine_barrier()
                _emit(nc, tc, pools, drams)

    nc.compile()
    return nc


def make_in_maps(inputs):
    bf = ml_dtypes.bfloat16
    x = np.asarray(inputs["x"], dtype=np.float32).reshape(B, C, HW).astype(bf)
    diff = (np.asarray(inputs["diff"], dtype=np.float32)
            .reshape(B, C, HW).astype(bf))
    # fold avg-pool 1/64 into the weights; append bias row (K=65 aug trick)
    inv = 1.0 / (DS * DS)
    wq_aug = np.concatenate(
        [np.asarray(inputs["Wq"]).T * inv, np.asarray(inputs["bq"])[None, :]], 0)
    wk_aug = np.concatenate(
        [np.asarray(inputs["Wk"]).T * inv, np.asarray(inputs["bk"])[None, :]], 0)
    wvt = np.asarray(inputs["Wv"]).T * inv
    wv2 = np.concatenate([wvt, wvt], 0)      # duplicated per s-half partition
    wq_aug = np.ascontiguousarray(wq_aug, dtype=np.float32).astype(bf)
    wk_aug = np.ascontiguousarray(wk_aug, dtype=np.float32).astype(bf)
    wv2 = np.ascontiguousarray(wv2, dtype=np.float32).astype(bf)
    bv = (np.ascontiguousarray(np.asarray(inputs["bv"])[None, :],
                               dtype=np.float32).astype(bf))
    return [
        {
            "x": x[b],
            "diff": diff[b],
            "wq": wq_aug, "wk": wk_aug, "wv": wv2, "bv": bv,
        }
        for b in range(B)
    ]


def kernel(x, diff, Wq, bq, Wk, bk, Wv, bv):
    if "nc" not in _cache:
        _cache["nc"] = _build()
    nc = _cache["nc"]

    in_maps = make_in_maps(dict(x=x, diff=diff, Wq=Wq, bq=bq, Wk=Wk, bk=bk,
                                Wv=Wv, bv=bv))
    res = run_bass_kernel_spmd(nc, in_maps, list(range(B)))
    out = np.stack([np.asarray(res.results[b]["out"])
                    .astype(np.float32).reshape(C, H, W) for b in range(B)])
    return out


if __name__ == "__main__":
    rng = np.random.default_rng(0)
    xs = rng.standard_normal((B, C, H, W), dtype=np.float32)
    ds = rng.standard_normal((B, C, H, W), dtype=np.float32)
    sc = 1.0 / np.sqrt(C)
    args = dict(
        x=xs, diff=ds,
        Wq=rng.standard_normal((C, C), dtype=np.float32) * sc,
        bq=rng.standard_normal(C, dtype=np.float32) * 0.01,
        Wk=rng.standard_normal((C, C), dtype=np.float32) * sc,
        bk=rng.standard_normal(C, dtype=np.float32) * 0.01,
        Wv=rng.standard_normal((C, C), dtype=np.float32) * sc,
        bv=rng.standard_normal(C, dtype=np.float32) * 0.01,
    )
    out = kernel(**args)
    print("kernel ran, out shape", out.shape, out.dtype)
